# revision 20
# baseline (speedup 1.0000x reference)
"""CrossAttention kernel for 8 Trainium2 NeuronCores.

Sharding: batch (4) x query-row-half (2) -> 8 shards, one per core. Each core
computes the full cross-attention for its 1024 query rows of one batch:
Q/K/V projections, 8 heads of attention, and the output projection. K/V
projections are recomputed by both cores sharing a batch (20% extra flops)
in exchange for zero collectives and a pure-SPMD single NEFF.

Layout trick: x and context are transposed (and cast to bf16) on the host so
the contraction dim lands on SBUF partitions with contiguous DMAs; all device
matmuls run without on-chip transposes:
  QT = Wq.T @ xT      (i on partitions)     KT = Wk.T @ ctxT
  V  = ctxT.T @ Wv    (natural [nk, i])
  ST_h = KT_h @ QT_h  ([nk, nq], K=64, head pairs in PE row groups)
  P = exp(ST * scale) (no max-subtraction; logits are ~N(0,1), safe range)
  O^T_h | den_h = [V_h | ones].T @ P  (denominator rides free in the M dim)
  Y = (O^T/den).T @ Wo + bo

Schedule: the attention phase is ACT-(exp)-bound while the projections are
pure TensorE work, so projection matmuls are interleaved into the attention
loop as PE filler between score groups. This keeps the PE HAM-warm and makes
total span ~= max(engine busy) instead of sum(phases). The softmax
denominator is normalized fully on-chip: den row (PSUM) -> SBUF via a tiny
DMA, reciprocal on DVE, partition-broadcast via a rank-2 matmul against a
constant 0/1 mask, multiply on DVE. nq chunks run c-outer so the first half
of the output projection drains mid-kernel.
"""

import numpy as np

HEADS = 8
DIM_HEAD = 64
SCALE = DIM_HEAD ** -0.5
B, NQ, DQ = 4, 2048, 512
NK, DC = 1024, 768
INNER = HEADS * DIM_HEAD  # 512
NQH = NQ // 2             # query rows per core
N_CORES = 8
P = 128

_PROG_CACHE = {}


def _build_program():
    import concourse.bacc as bacc
    import concourse.tile as tile
    from concourse import mybir
    from concourse.bass import ts, ds

    f32 = mybir.dt.float32
    f32r = mybir.dt.float32r
    bf16 = mybir.dt.bfloat16
    Exp = mybir.ActivationFunctionType.Exp

    nc = bacc.Bacc(
        "TRN2",
        target_bir_lowering=False,
        debug=False,
        num_devices=N_CORES,
    )

    xT_d = nc.dram_tensor("xT", [DQ, NQH], bf16, kind="ExternalInput")
    ctxT_d = nc.dram_tensor("ctxT", [DC, NK], bf16, kind="ExternalInput")
    Wq_d = nc.dram_tensor("Wq", [DQ, INNER], bf16, kind="ExternalInput")
    Wk_d = nc.dram_tensor("Wk", [DC, INNER], bf16, kind="ExternalInput")
    Wv_d = nc.dram_tensor("Wv", [DC, INNER], bf16, kind="ExternalInput")
    Wo_d = nc.dram_tensor("Wo", [INNER, DQ], bf16, kind="ExternalInput")
    bo_d = nc.dram_tensor("bo", [DQ], f32, kind="ExternalInput")
    ones_d = nc.dram_tensor("ones", [4, 128], bf16, kind="ExternalInput")
    diag_d = nc.dram_tensor("diag2", [P, 128], f32r, kind="ExternalInput")
    zr_d = nc.dram_tensor("zr", [P, 512], f32r, kind="ExternalInput")
    Y_d = nc.dram_tensor("Y", [NQH, DQ], f32, kind="ExternalOutput")

    KQ = DQ // P      # 4  k-tiles for x-side contraction
    KC = DC // P      # 6  k-tiles for context-side contraction
    KI = INNER // P   # 4  k-tiles for inner-dim contraction
    NQT = NQH // P    # 8  query row tiles
    NKT = NK // P     # 8  key row tiles
    NCH = NQH // 512  # 2  nq chunks of 512

    with tile.TileContext(nc) as tc:
        with (
            tc.tile_pool(name="consts", bufs=1) as consts,
            tc.tile_pool(name="sc", bufs=2, space="PSUM") as scp,
            tc.tile_pool(name="pv", bufs=2, space="PSUM") as pvp,
            tc.tile_pool(name="mm", bufs=2, space="PSUM") as mmp,
            tc.tile_pool(name="ep", bufs=16) as ep,
            tc.tile_pool(name="yp", bufs=2) as yp,
        ):
            # ---- staged inputs; scalar queue carries the critical first deps
            Wk_sb = consts.tile([P, KC, INNER], bf16, tag="wk")
            Wq_sb = consts.tile([P, KQ, INNER], bf16, tag="wq")
            Wv_sb = consts.tile([P, KC, INNER], bf16, tag="wv")
            Wo_sb = consts.tile([P, KI, DQ], bf16, tag="wo")
            ctx_sb = consts.tile([P, KC, NK], bf16, tag="ctx")
            xT_sb = consts.tile([P, KQ, NQH], bf16, tag="x")
            bo_sb = consts.tile([P, DQ], f32, tag="bo")
            diag_sb = consts.tile([P, P], f32r, tag="diag")
            # 1/den rows land at partitions 0 (odd head) / 64 (even head);
            # all other partitions must read as exact zeros for the
            # mask-matmul broadcast, so clear once up front via DMA
            # (memset can't write f32r).
            r_sb = consts.tile([P, 512], f32r, tag="recip")
            nc.gpsimd.dma_start(out=r_sb, in_=zr_d.ap())

            nc.scalar.dma_start(
                out=Wk_sb, in_=Wk_d.ap().rearrange("(ko p) i -> p ko i", p=P)
            )
            ctx_src = ctxT_d.ap().rearrange("(ko p) n -> p ko n", p=P)
            for k in range(KC):
                nc.sync.dma_start(
                    out=ctx_sb[:, k:k + 1, :], in_=ctx_src[:, k:k + 1, :]
                )
            nc.scalar.dma_start(
                out=Wq_sb, in_=Wq_d.ap().rearrange("(ko p) i -> p ko i", p=P)
            )
            xT_src = xT_d.ap().rearrange("(ko p) n -> p ko n", p=P)
            for k in range(KQ):  # c0 halves first: S(0,0) needs them
                nc.scalar.dma_start(
                    out=xT_sb[:, k, 0:512], in_=xT_src[:, k, 0:512]
                )
            nc.sync.dma_start(
                out=Wv_sb, in_=Wv_d.ap().rearrange("(ko p) i -> p ko i", p=P)
            )
            for k in range(KQ):
                nc.sync.dma_start(
                    out=xT_sb[:, k, 512:1024], in_=xT_src[:, k, 512:1024]
                )
            nc.sync.dma_start(
                out=Wo_sb, in_=Wo_d.ap().rearrange("(ko p) i -> p ko i", p=P)
            )
            nc.sync.dma_start(
                out=bo_sb, in_=bo_d.ap().unsqueeze(0).to_broadcast((P, DQ))
            )
            nc.gpsimd.dma_start(out=diag_sb, in_=diag_d.ap())

            KT_sb = consts.tile([P, KI, NK], bf16, tag="kt")    # [i, nk]
            QT_sb = consts.tile([P, KI, NQH], bf16, tag="qt")   # [i, nq]
            # V in natural [nk, i] layout padded per head to 128 cols:
            # even head h: cols h*128+[0:64]=V_h, [64:128]=ones
            # odd  head h: cols h*128+[0:64]=ones, [64:128]=V_h
            V_sb = consts.tile([P, NKT, HEADS * P], bf16, tag="v")
            OT_sb = consts.tile([P, KI, NQH], bf16, tag="ot")   # [i, nq]

            ones_src = ones_d.ap().unsqueeze(0).to_broadcast((P, 4, 128))
            for t in range(NKT):
                dv4 = V_sb[:, t, :].rearrange("p (j y) -> p j y", j=4)
                nc.gpsimd.dma_start(out=dv4[:, :, 64:192], in_=ones_src)

            # ---- PE work units (emitted lazily as schedule filler) ----
            def kp_unit(m, c):  # K projection: KT[:, m, c*512:...]
                psk = mmp.tile([P, 512], f32, tag="acc")
                for k in range(KC):
                    nc.tensor.matmul(
                        psk,
                        lhsT=Wk_sb[:, k, ts(m, P)],
                        rhs=ctx_sb[:, k, ds(c * 512, 512)],
                        start=(k == 0),
                        stop=(k == KC - 1),
                    )
                nc.vector.tensor_copy(KT_sb[:, m, ds(c * 512, 512)], psk)

            def qp_unit(m, c):  # Q projection: QT[:, m, c*512:...]
                psq = mmp.tile([P, 512], f32, tag="acc")
                for k in range(KQ):
                    nc.tensor.matmul(
                        psq,
                        lhsT=Wq_sb[:, k, ts(m, P)],
                        rhs=xT_sb[:, k, ds(c * 512, 512)],
                        start=(k == 0),
                        stop=(k == KQ - 1),
                    )
                nc.vector.tensor_copy(QT_sb[:, m, ds(c * 512, 512)], psq)

            def vp_unit(t):  # V projection tile t, scattered into head pads
                psv = mmp.tile([P, 512], f32, tag="acc")
                for k in range(KC):
                    nc.tensor.matmul(
                        psv,
                        lhsT=ctx_sb[:, k, ts(t, P)],
                        rhs=Wv_sb[:, k, :],
                        start=(k == 0),
                        stop=(k == KC - 1),
                    )
                pv4 = psv.rearrange("p (j x) -> p j x", j=4)
                dv4 = V_sb[:, t, :].rearrange("p (j y) -> p j y", j=4)
                nc.vector.tensor_copy(dv4[:, :, 0:64], pv4[:, :, 0:64])
                nc.vector.tensor_copy(dv4[:, :, 192:256], pv4[:, :, 64:128])

            def op_unit(m):  # output projection row tile m
                psy = mmp.tile([P, 512], f32, tag="acc")
                for k in range(KI):
                    nc.tensor.matmul(
                        psy,
                        lhsT=OT_sb[:, k, ts(m, P)],
                        rhs=Wo_sb[:, k, :],
                        start=(k == 0),
                        stop=(k == KI - 1),
                    )
                y_t = yp.tile([P, DQ], f32, tag="y")
                nc.vector.tensor_tensor(y_t, psy, bo_sb, op=mybir.AluOpType.add)
                eng = nc.sync if m % 2 == 0 else nc.gpsimd
                eng.dma_start(out=Y_d.ap()[ts(m, P), :], in_=y_t)

            # ---- pre-loop: projections needed by the first score groups ----
            kp_unit(0, 0)
            kp_unit(0, 1)
            qp_unit(0, 0)
            qp_unit(0, 1)

            # ---- attention steps, c-outer so out-proj c0 drains early ----
            steps = [(j, c) for c in range(NCH) for j in range(HEADS // 2)]
            fills = {
                0: [lambda: kp_unit(1, 0), lambda: kp_unit(1, 1),
                    lambda: qp_unit(1, 0), lambda: vp_unit(0),
                    lambda: vp_unit(1), lambda: vp_unit(2)],
                1: [lambda: vp_unit(3), lambda: vp_unit(4),
                    lambda: vp_unit(5), lambda: vp_unit(6),
                    lambda: vp_unit(7), lambda: kp_unit(2, 0),
                    lambda: kp_unit(2, 1), lambda: qp_unit(2, 0)],
                2: [lambda: kp_unit(3, 0), lambda: kp_unit(3, 1),
                    lambda: qp_unit(3, 0), lambda: qp_unit(1, 1)],
                3: [lambda: qp_unit(2, 1), lambda: qp_unit(3, 1)],
                4: [],
                5: [lambda: op_unit(0), lambda: op_unit(1)],
                6: [lambda: op_unit(2), lambda: op_unit(3)],
                7: [],
            }

            def s_group(j, c, t):  # one nk-tile of scores for head pair j
                ps_g = scp.tile([P, 2, 512], f32, tag="s")
                e_g = ep.tile([P, 2, 512], bf16, tag="e")
                nc.tensor.matmul(
                    ps_g[:, 0, :],
                    lhsT=KT_sb[0:64, j, ts(t, P)],
                    rhs=QT_sb[0:64, j, ds(c * 512, 512)],
                    start=True, stop=True,
                )
                nc.tensor.matmul(
                    ps_g[:, 1, :],
                    lhsT=KT_sb[64:128, j, ts(t, P)],
                    rhs=QT_sb[64:128, j, ds(c * 512, 512)],
                    start=True, stop=True,
                )
                nc.scalar.activation(out=e_g, in_=ps_g, func=Exp, scale=SCALE)
                return e_g

            def pv_chain(j, c, e_gs):
                """PV for both heads of pair j; returns norm finisher."""
                pos = []
                for ab, h in enumerate((2 * j, 2 * j + 1)):
                    po = pvp.tile([P, 512], f32, tag="po")
                    pos.append(po)
                    for t in range(NKT):
                        nc.tensor.matmul(
                            po,
                            lhsT=V_sb[:, t, ds(h * P, P)],
                            rhs=e_gs[t][:, ab, :],
                            start=(t == 0),
                            stop=(t == NKT - 1),
                        )
                    # den copies sit on the ones-columns' partitions; take
                    # one row each, reciprocal straight out of PSUM at the
                    # same partition (64 for even head, 0 for odd head)
                    dpart = 64 if ab == 0 else 0
                    with nc.allow_low_precision(
                        reason="1/den feeds an f32r matmul; f32r is ample"
                    ):
                        nc.vector.reciprocal(
                            r_sb[dpart:dpart + 1, :], po[dpart:dpart + 1, :]
                        )

                def finish():
                    ps_rb = mmp.tile([P, 512], f32, tag="acc")
                    nc.tensor.matmul(
                        ps_rb, lhsT=diag_sb, rhs=r_sb, start=True, stop=True
                    )
                    # DVE may read only one PSUM operand per instruction
                    rb_sb = yp.tile([P, 512], f32, tag="rb")
                    nc.vector.tensor_copy(rb_sb, ps_rb)
                    csl = ds(c * 512, 512)
                    nc.vector.tensor_tensor(
                        OT_sb[0:64, j, csl], pos[0][0:64, :], rb_sb[0:64, :],
                        op=mybir.AluOpType.mult,
                    )
                    nc.vector.tensor_tensor(
                        OT_sb[64:128, j, csl], pos[1][64:128, :],
                        rb_sb[64:128, :], op=mybir.AluOpType.mult,
                    )
                return finish

            prev = None          # (j, c, e_groups) awaiting PV
            pending_norm = None  # rank-2 broadcast + mults of prior PV
            for i, (j, c) in enumerate(steps):
                filler = list(fills[i])
                e_gs = []
                for t in range(NKT):
                    e_gs.append(s_group(j, c, t))
                    # norm of the PV two steps back: must precede any filler
                    # that reads OT (out-proj), hence t==1 before pops at t>=2
                    if t == 1 and pending_norm is not None:
                        pending_norm()
                        pending_norm = None
                    if t >= 2 and filler:
                        filler.pop(0)()
                for f in filler:
                    f()
                if prev is not None:
                    pending_norm = pv_chain(*prev)
                prev = (j, c, e_gs)

            pending_norm()
            fin = pv_chain(*prev)
            fin()
            for m in range(4, NQT):
                op_unit(m)

    nc.finalize()
    return nc


def _get_program():
    if "nc" not in _PROG_CACHE:
        _PROG_CACHE["nc"] = _build_program()
    return _PROG_CACHE["nc"]


def _consts():
    import ml_dtypes
    ones = np.ones((4, 128), dtype=ml_dtypes.bfloat16)
    # mask for the 1/den partition broadcast: contraction row 64 carries the
    # even head's reciprocal (-> out partitions 0:64 where its O rows live),
    # row 0 carries the odd head's (-> out partitions 64:128)
    diag = np.zeros((128, 128), dtype=np.float32)
    diag[64, 0:64] = 1.0
    diag[0, 64:128] = 1.0
    zr = np.zeros((128, 512), dtype=np.float32)
    return ones, diag, zr


def kernel(x, context, Wq, Wk, Wv, Wo, bo, **_unused):
    import ml_dtypes
    from concourse.bass_utils import run_bass_kernel_spmd

    bf = ml_dtypes.bfloat16
    x = np.asarray(x, dtype=np.float32)
    context = np.asarray(context, dtype=np.float32)
    Wqb = np.ascontiguousarray(np.asarray(Wq, dtype=np.float32).astype(bf))
    Wkb = np.ascontiguousarray(np.asarray(Wk, dtype=np.float32).astype(bf))
    Wvb = np.ascontiguousarray(np.asarray(Wv, dtype=np.float32).astype(bf))
    Wob = np.ascontiguousarray(np.asarray(Wo, dtype=np.float32).astype(bf))
    bo = np.ascontiguousarray(np.asarray(bo, dtype=np.float32))
    ones, diag, zr = _consts()

    nc = _get_program()
    in_maps = []
    for core in range(N_CORES):
        b, half = divmod(core, 2)
        xs = np.ascontiguousarray(
            x[b, half * NQH:(half + 1) * NQH, :].T.astype(bf)
        )
        cs = np.ascontiguousarray(context[b].T.astype(bf))
        in_maps.append(
            {"xT": xs, "ctxT": cs, "Wq": Wqb, "Wk": Wkb, "Wv": Wvb,
             "Wo": Wob, "bo": bo, "ones": ones, "diag2": diag, "zr": zr}
        )

    res = run_bass_kernel_spmd(nc, in_maps, core_ids=list(range(N_CORES)))

    out = np.empty((B, NQ, DQ), np.float32)
    for core in range(N_CORES):
        b, half = divmod(core, 2)
        out[b, half * NQH:(half + 1) * NQH, :] = res.results[core]["Y"]
    return out
